# revision 31
# baseline (speedup 1.0000x reference)
"""Trainium2 Bass kernel for nn_CSABlock (dual spatial-attention gating).

Reference computation:
    sa_x  = sigmoid(conv3d(concat[max_c(x), mean_c(x)], w, k=7, pad=3))
    fix_out  = move * sa_fix + fix
    move_out = fix * sa_move + move

Sharding: 8 cores = (batch 2) x (D quarters of 20 planes). Each core gets a
26-plane input slab (3-voxel halo each side) per tensor in bf16 and produces
20 output planes in bf16; the host casts f32<->bf16 and shards/gathers.

v6 design notes (evidence from the v5 trace):
  - v5 ran DVE and GpSimd tensor_tensor streams concurrently. They arbitrate
    an exclusive SBUF shared-port lock, so they serialize: DVE ops overlapped
    by GpSimd measured 3.0x their cost-model time vs 1.07x when alone.
    v6 therefore runs ALL elementwise work on DVE (2 elem/cyc bf16) and
    leaves GpSimd idle; ACT only does memzero + sigmoid.
  - fix/move ride one extra tensor dim everywhere (loads, trees, stats,
    gates, gating, stores), halving instruction and DMA counts.
  - All DMA is HWDGE. sync queue: loads, P-stage, stores (issue order
    matches semaphore resolution order, so no head-of-line stalls).
    scalar queue: weights + gate reshapes (right after their sigmoids).
  - Dummy matmuls on the weight tile bridge the PE-idle gap before the
    first conv group so HAM doesn't drop the PE clock to 1.2 GHz.

Per-core pipeline:
  - Stream D in chunks (3,4,4,4,4,4,3 planes). Load tile layout:
    partition=(d,hg32), free=(t2, c16, hp3*w96) bf16 -> 576B lines.
  - Channel max/sum trees on DVE (mean's 1/16 folded into conv weights);
    final tree level writes fp8 stats.
  - Stats staged into persistent P [hin_pad128, t2, stat2, dp28, wp102]
    fp8 via per-d SBUF->SBUF reshape DMAs.
  - Conv: 49 fp8 DoubleRow matmuls per 4-plane output group per tensor;
    stats ride the k-tile dim; contraction over padded H with banded
    weights lhsT[hin, t, (kd,kw), c, hout] scaled by WS=256.
  - Sigmoid on ACT with scale=1/WS (PSUM -> SBUF bf16), reshaped to the
    data layout via 4 small DMAs per group.
  - Gating all-bf16 on DVE, one 16-channel op per (tensor, mul/add).
"""

import sys

import numpy as np

for _p in ("/opt/trn_rl_repo",):
    if _p not in sys.path:
        sys.path.insert(0, _p)

import ml_dtypes  # noqa: E402

B, C, D, H, W = 2, 16, 80, 96, 96
KK = 7
DSLAB = 28          # slab plane indexing (plane 0 and 27 never loaded)
OUTD = 20           # output planes per core
HG, HPW = 32, 3     # h = hg*3 + hp
WPAD = 104          # 102 needed; padded to /4 so ACT memzero can bitcast u32
NPAIR = KK * KK     # 49 DoubleRow matmuls per output group
NOC = 5             # output groups of G=4 planes
G = 4
NCORES = 8
WS = 256.0          # fp8 weight scale; undone in the sigmoid
CHUNKS = [(1, 3), (4, 4), (8, 4), (12, 4), (16, 4), (20, 4), (24, 3)]
LOADD = 26          # planes 1..26 inclusive
NWARM = 96          # PE warm-up dummy matmuls before conv group 0

_prog_cache: dict = {}


def _build_banded(w: np.ndarray) -> np.ndarray:
    """w: [1,2,7,7,7] f32 -> lhsT [hin_pad 128, pair 49, c 2, hout 96] f32.

    out[o,h,w'] = sum_{c,kd,kw} lhsT[hq, kd*7+kw, c, h] * P[hq, c, o+1+kd, w'+kw]
    with P[h_in+3, stat, dp, w_in+3] = pooled stats (0=max, 1=sum) and
    lhsT[h+kh, kd*7+kw, c, h] = w[c,kd,kh,kw] * WS * (1/16 for c=1).
    """
    A = np.zeros((128, NPAIR, 2, 96), np.float32)
    hh = np.arange(96)
    for c in range(2):
        scale = WS if c == 0 else WS / C
        for kd in range(KK):
            for kw in range(KK):
                pair = kd * KK + kw
                for kh in range(KK):
                    A[hh + kh, pair, c, hh] = w[0, c, kd, kh, kw] * scale
    return A


def _build_program():
    import concourse.bass as bass  # noqa: F401
    import concourse.bacc as bacc
    import concourse.tile as tile
    from concourse import mybir
    from contextlib import ExitStack

    bf16 = mybir.dt.bfloat16
    fp8 = mybir.dt.float8e4

    nc = bacc.Bacc("TRN2")
    # xin/xout are host-pre-shuffled to the on-chip tile layout
    # (d, hg, t, c, hp, w) so loads and stores are fully contiguous
    # (~128 fat descriptors instead of 4096x576B -> ~10x cheaper
    # HWDGE issue and full line rate).
    xin = nc.dram_tensor(
        "xin", [LOADD, HG, 2, C, HPW, W], bf16, kind="ExternalInput"
    )
    wgt = nc.dram_tensor("wgt", [128, 2, NPAIR, 2, 96], fp8, kind="ExternalInput")
    xout = nc.dram_tensor(
        "xout", [OUTD, HG, 2, C, HPW, W], bf16, kind="ExternalOutput"
    )

    with tile.TileContext(nc) as tc, ExitStack() as ctx:
        singles = ctx.enter_context(tc.tile_pool(name="singles", bufs=1))
        lp = ctx.enter_context(tc.tile_pool(name="lp", bufs=5))
        trpool = ctx.enter_context(tc.tile_pool(name="tr", bufs=2))
        pspool = ctx.enter_context(tc.tile_pool(name="pstage", bufs=2))
        tpool = ctx.enter_context(tc.tile_pool(name="tmp", bufs=1))
        gpool = ctx.enter_context(tc.tile_pool(name="gate", bufs=3))
        gtpool = ctx.enter_context(tc.tile_pool(name="gateT", bufs=3))
        psum = ctx.enter_context(tc.tile_pool(name="psum", bufs=4, space="PSUM"))

        WG = singles.tile([128, 2, NPAIR, 2, 96], fp8)
        # Persistent pooled stats [hin_pad, tensor, stat(max,sum), dp, wp]
        P = singles.tile([128, 2, 2, DSLAB, WPAD], fp8)

        # ACT zeroes P (cheap, ACT is idle early; GpSimd memset would
        # grab the DVE-shared SBUF port). Weights load right behind on
        # the scalar ring; the sync ring drains input loads.
        nc.scalar.memzero(P[:].rearrange("p t s d w -> p t s (d w)"))
        nc.scalar.dma_start(out=WG[:], in_=wgt[:])

        ltiles: dict = {}

        def load_chunk(ic: int, split=False):
            p0, nd = CHUNKS[ic]
            np_ = nd * HG
            L = lp.tile([128, 2, C, HPW * W], bf16, tag="L")
            if split:  # two DMAs so tensor-f trees can start sooner
                for t in range(2):
                    src = xin[p0 - 1:p0 - 1 + nd, :, t].rearrange(
                        "d hg c hp w -> (d hg) c (hp w)"
                    )
                    nc.sync.dma_start(out=L[:np_, t], in_=src)
            else:
                src = xin[p0 - 1:p0 - 1 + nd].rearrange(
                    "d hg t c hp w -> (d hg) t c (hp w)"
                )
                nc.sync.dma_start(out=L[:np_], in_=src)
            ltiles[ic] = L

        def trees(ic: int):
            """Channel max+sum trees for both tensors on DVE, then stage
            the fp8 stats into P with one reshape DMA per plane."""
            p0, nd = CHUNKS[ic]
            np_ = nd * HG
            Lv = ltiles[ic][:np_]
            Tmax = trpool.tile([128, 2, 8, HPW * W], bf16, tag="Tmax")
            Tsum = trpool.tile([128, 2, 8, HPW * W], bf16, tag="Tsum")
            PS = pspool.tile([128, 2, 2, HPW, W], fp8, tag="PS")
            v = nc.vector
            v.tensor_max(Tmax[:np_], Lv[:, :, 0:8, :], Lv[:, :, 8:16, :])
            v.tensor_add(Tsum[:np_], Lv[:, :, 0:8, :], Lv[:, :, 8:16, :])
            v.tensor_max(Tmax[:np_, :, 0:4], Tmax[:np_, :, 0:4], Tmax[:np_, :, 4:8])
            v.tensor_add(Tsum[:np_, :, 0:4], Tsum[:np_, :, 0:4], Tsum[:np_, :, 4:8])
            v.tensor_max(Tmax[:np_, :, 0:2], Tmax[:np_, :, 0:2], Tmax[:np_, :, 2:4])
            v.tensor_add(Tsum[:np_, :, 0:2], Tsum[:np_, :, 0:2], Tsum[:np_, :, 2:4])
            PSm = PS[:np_, :, 0].rearrange("p t hp w -> p t (hp w)")
            PSs = PS[:np_, :, 1].rearrange("p t hp w -> p t (hp w)")
            v.tensor_max(PSm, Tmax[:np_, :, 0], Tmax[:np_, :, 1])
            v.tensor_add(PSs, Tsum[:np_, :, 0], Tsum[:np_, :, 1])
            return PS

        def trees_half(ic: int, t: int, Tmax, Tsum, PS):
            """One tensor's tree for one chunk. The f halves of chunks
            0-1 run before any m half so Pf(0,1) stages early and the
            first conv passes (kd 0-3 only read planes 1-7) start ~15us
            sooner."""
            p0, nd = CHUNKS[ic]
            np_ = nd * HG
            Lt = ltiles[ic][:np_, t]
            Tm, Ts = Tmax[:np_, t], Tsum[:np_, t]
            v = nc.vector
            v.tensor_max(Tm, Lt[:, 0:8, :], Lt[:, 8:16, :])
            v.tensor_add(Ts, Lt[:, 0:8, :], Lt[:, 8:16, :])
            v.tensor_max(Tm[:, 0:4], Tm[:, 0:4], Tm[:, 4:8])
            v.tensor_add(Ts[:, 0:4], Ts[:, 0:4], Ts[:, 4:8])
            v.tensor_max(Tm[:, 0:2], Tm[:, 0:2], Tm[:, 2:4])
            v.tensor_add(Ts[:, 0:2], Ts[:, 0:2], Ts[:, 2:4])
            PSm = PS[:np_, t, 0].rearrange("p hp w -> p (hp w)")
            PSs = PS[:np_, t, 1].rearrange("p hp w -> p (hp w)")
            v.tensor_max(PSm, Tm[:, 0], Tm[:, 1])
            v.tensor_add(PSs, Ts[:, 0], Ts[:, 1])

        def pstage_half(ic: int, t: int, PS):
            p0, nd = CHUNKS[ic]
            for d in range(nd):
                for hp in range(HPW):
                    eng = (nc.sync, nc.scalar)[(d * HPW + hp) % 2]
                    eng.dma_start(
                        out=P[3 + hp:3 + hp + 94:3, t, :, p0 + d, 3:3 + W],
                        in_=PS[d * HG:(d + 1) * HG, t, :, hp, :],
                    )

        def pstage(ic: int, PS, engines=(nc.sync, nc.scalar)):
            # stage into P: per (plane, hp): src partitions d*32+hg, free
            # (t,s,w); dst partitions 3+hp+3*hg (step 3). Both sides merge
            # (t,s) so the DMA balancer sees 3 dims. Early chunks split
            # across both HWDGE queues (issue ~0.7us each); chunks 4-6 go
            # sync-only so gate reshapes never queue behind P batches on
            # the scalar ring (that head-of-line starved DVE ~18us/group).
            p0, nd = CHUNKS[ic]
            for d in range(nd):
                for hp in range(HPW):
                    eng = engines[(d * HPW + hp) % len(engines)]
                    eng.dma_start(
                        out=P[3 + hp:3 + hp + 94:3, :, :, p0 + d, 3:3 + W],
                        in_=PS[d * HG:(d + 1) * HG, :, :, hp, :],
                    )

        def conv_group(oc: int):
            o0 = G * oc
            gate = gpool.tile([96, 2, G, 96], bf16, tag="gate")
            gateT = gtpool.tile([128, 2, HPW, W], bf16, tag="gT")
            for t in range(2):
                acc = psum.tile([96, G, 96], mybir.dt.float32, tag="acc")
                for kd in range(KK):
                    dsl = slice(o0 + 1 + kd, o0 + 1 + kd + G)
                    for kw in range(KK):
                        nc.tensor.matmul(
                            acc[:],
                            WG[:, t, kd * KK + kw],
                            P[:, t, 0:2, dsl, kw:kw + 96],
                            start=(kd == 0 and kw == 0),
                            stop=(kd == KK - 1 and kw == KK - 1),
                            perf_mode=mybir.MatmulPerfMode.DoubleRow,
                        )
                nc.scalar.activation(
                    out=gate[:, t], in_=acc[:],
                    func=mybir.ActivationFunctionType.Sigmoid,
                    scale=1.0 / WS,
                )
                # reshape this tensor's gate immediately: gating's first
                # ops need only the f half, so they overlap the m conv
                for d in range(G):
                    nc.scalar.dma_start(
                        out=gateT[d * HG:(d + 1) * HG, t],
                        in_=gate[:, t, d, :],
                    )
            return gateT

        def gating(oc: int, gateT):
            L = ltiles[oc + 1]
            gf = (
                gateT[:, 0].rearrange("p hp w -> p (hp w)").unsqueeze(1)
                .broadcast_to((128, C, HPW * W))
            )
            gm = (
                gateT[:, 1].rearrange("p hp w -> p (hp w)").unsqueeze(1)
                .broadcast_to((128, C, HPW * W))
            )
            # products into a temp, then add in place into L: fo lands in
            # the fix slot, mo in the move slot; the store reads L itself.
            # Per-tensor stores so the fo store overlaps the mo add.
            T = tpool.tile([128, 2, C, HPW * W], bf16, tag="T")
            v = nc.vector
            # broadcast operand in slot 0 (slot-1 broadcast is ~2x slower)
            v.tensor_mul(T[:, 0], gf, L[:, 1])
            v.tensor_mul(T[:, 1], gm, L[:, 0])
            dst = xout[G * oc:G * oc + G]
            v.tensor_add(L[:, 0], T[:, 0], L[:, 0])
            nc.sync.dma_start(
                out=dst[:, :, 0].rearrange("d hg c hp w -> (d hg) c (hp w)"),
                in_=L[:, 0],
            )
            v.tensor_add(L[:, 1], T[:, 1], L[:, 1])
            nc.sync.dma_start(
                out=dst[:, :, 1].rearrange("d hg c hp w -> (d hg) c (hp w)"),
                in_=L[:, 1],
            )

        # Software pipeline. DVE queue order: t0..t5, g0, t6, g1..g4
        # (all trees lead their gating consumers so P staging never
        # trails the PE and the cross-engine gate waits never
        # head-of-line block tree work). Sync queue issue order is
        # monotone in semaphore-resolution time:
        #   L0-3, P0[t0], P1[t1], L4, P2[t2], L5[t0/buf], P3[t3],
        #   P4[t4], P5[t5], S0[g0], L6[g0/buf], P6[t6], S1[g1], S2-4.
        load_chunk(0, split=True)
        load_chunk(1, split=True)
        load_chunk(2)
        load_chunk(3)
        # chunk-0 temps from trpool; chunk-1 temps borrow the gating temp
        # buffer (tpool is otherwise unused until gating(0) at ~80us)
        tm0 = trpool.tile([128, 2, 8, HPW * W], bf16, tag="Tmax")
        ts0 = trpool.tile([128, 2, 8, HPW * W], bf16, tag="Tsum")
        ps0 = pspool.tile([128, 2, 2, HPW, W], fp8, tag="PS")
        bor = tpool.tile([128, 2, C, HPW * W], bf16, tag="T")
        tm1 = bor[:, :, 0:8, :]
        ts1 = bor[:, :, 8:16, :]
        ps1 = pspool.tile([128, 2, 2, HPW, W], fp8, tag="PS")
        trees_half(0, 0, tm0, ts0, ps0)
        trees_half(1, 0, tm1, ts1, ps1)
        pstage_half(0, 0, ps0)
        pstage_half(1, 0, ps1)
        trees_half(0, 1, tm0, ts0, ps0)
        trees_half(1, 1, tm1, ts1, ps1)
        pstage_half(0, 1, ps0)
        pstage_half(1, 1, ps1)
        load_chunk(4)
        ps2 = trees(2)
        pstage(2, ps2)
        load_chunk(5)  # reuses chunk 0's buffer (freed after trees 0)
        ps3 = trees(3)
        pstage(3, ps3)
        gts = {0: conv_group(0)}
        ps4 = trees(4)
        pstage(4, ps4, engines=(nc.sync,))
        ps5 = trees(5)
        pstage(5, ps5, engines=(nc.sync,))
        gating(0, gts[0])
        load_chunk(6)  # reuses chunk 1's buffer; gating(0) read it above
        gts[1] = conv_group(1)
        ps6 = trees(6)
        pstage(6, ps6, engines=(nc.sync,))
        gating(1, gts[1])
        gts[2] = conv_group(2)
        gating(2, gts[2])
        gts[3] = conv_group(3)
        gating(3, gts[3])
        gts[4] = conv_group(4)
        gating(4, gts[4])

    nc.compile()
    return nc


def _get_program():
    if "nc" not in _prog_cache:
        _prog_cache["nc"] = _build_program()
    return _prog_cache["nc"]


def _shard(fix, move, Wm):
    in_maps = []
    for core in range(NCORES):
        b, dq = core // 4, core % 4
        lo = 20 * dq - 3  # global index of slab plane 1
        s0, s1 = max(lo, 0), min(lo + LOADD, D)
        slab = np.zeros((2, C, LOADD, H, W), ml_dtypes.bfloat16)
        slab[0, :, s0 - lo:s1 - lo] = fix[b, :, s0:s1]
        slab[1, :, s0 - lo:s1 - lo] = move[b, :, s0:s1]
        # host-side shuffle to the on-chip layout (d, hg, t, c, hp, w)
        shuf = np.ascontiguousarray(
            slab.reshape(2, C, LOADD, HG, HPW, W).transpose(2, 3, 0, 1, 4, 5)
        )
        in_maps.append({"xin": shuf, "wgt": Wm})
    return in_maps


def kernel(fix, move, w_f2m, w_m2f, __trace=False):
    fix = np.asarray(fix, dtype=np.float32).astype(ml_dtypes.bfloat16)
    move = np.asarray(move, dtype=np.float32).astype(ml_dtypes.bfloat16)
    Af = _build_banded(np.asarray(w_f2m, dtype=np.float32))
    Am = _build_banded(np.asarray(w_m2f, dtype=np.float32))
    Wm = np.ascontiguousarray(
        np.stack([Af, Am]).transpose(1, 0, 2, 3, 4)
    ).astype(ml_dtypes.float8_e4m3fn)

    nc = _get_program()
    in_maps = _shard(fix, move, Wm)

    from concourse.bass_utils import run_bass_kernel_spmd

    res = run_bass_kernel_spmd(
        nc, in_maps, core_ids=list(range(NCORES)), trace=__trace
    )
    _prog_cache["last_results"] = res

    fix_out = np.empty((B, C, D, H, W), np.float32)
    move_out = np.empty((B, C, D, H, W), np.float32)
    for core in range(NCORES):
        b, dq = core // 4, core % 4
        out = res.results[core]["xout"]  # [d, hg, t, c, hp, w]
        out = out.transpose(2, 3, 0, 1, 4, 5).reshape(
            2, C, OUTD, H, W
        ).astype(np.float32)
        fix_out[b, :, 20 * dq:20 * dq + 20] = out[0]
        move_out[b, :, 20 * dq:20 * dq + 20] = out[1]
    return fix_out, move_out


# revision 32
# speedup vs baseline: 1.0639x; 1.0639x over previous
"""Trainium2 Bass kernel for nn_CSABlock (dual spatial-attention gating).

Reference computation:
    sa_x  = sigmoid(conv3d(concat[max_c(x), mean_c(x)], w, k=7, pad=3))
    fix_out  = move * sa_fix + fix
    move_out = fix * sa_move + move

Sharding: 8 cores = (batch 2) x (D quarters of 20 planes). Each core gets a
26-plane input slab (3-voxel halo each side) per tensor in bf16 and produces
20 output planes in bf16; the host casts f32<->bf16 and shards/gathers.

v6 design notes (evidence from the v5 trace):
  - v5 ran DVE and GpSimd tensor_tensor streams concurrently. They arbitrate
    an exclusive SBUF shared-port lock, so they serialize: DVE ops overlapped
    by GpSimd measured 3.0x their cost-model time vs 1.07x when alone.
    v6 therefore runs ALL elementwise work on DVE (2 elem/cyc bf16) and
    leaves GpSimd idle; ACT only does memzero + sigmoid.
  - fix/move ride one extra tensor dim everywhere (loads, trees, stats,
    gates, gating, stores), halving instruction and DMA counts.
  - All DMA is HWDGE. sync queue: loads, P-stage, stores (issue order
    matches semaphore resolution order, so no head-of-line stalls).
    scalar queue: weights + gate reshapes (right after their sigmoids).
  - Dummy matmuls on the weight tile bridge the PE-idle gap before the
    first conv group so HAM doesn't drop the PE clock to 1.2 GHz.

Per-core pipeline:
  - Stream D in chunks (3,4,4,4,4,4,3 planes). Load tile layout:
    partition=(d,hg32), free=(t2, c16, hp3*w96) bf16 -> 576B lines.
  - Channel max/sum trees on DVE (mean's 1/16 folded into conv weights);
    final tree level writes fp8 stats.
  - Stats staged into persistent P [hin_pad128, t2, stat2, dp28, wp102]
    fp8 via per-d SBUF->SBUF reshape DMAs.
  - Conv: 49 fp8 DoubleRow matmuls per 4-plane output group per tensor;
    stats ride the k-tile dim; contraction over padded H with banded
    weights lhsT[hin, t, (kd,kw), c, hout] scaled by WS=256.
  - Sigmoid on ACT with scale=1/WS (PSUM -> SBUF bf16), reshaped to the
    data layout via 4 small DMAs per group.
  - Gating all-bf16 on DVE, one 16-channel op per (tensor, mul/add).
"""

import sys

import numpy as np

for _p in ("/opt/trn_rl_repo",):
    if _p not in sys.path:
        sys.path.insert(0, _p)

import ml_dtypes  # noqa: E402

B, C, D, H, W = 2, 16, 80, 96, 96
KK = 7
DSLAB = 28          # slab plane indexing (plane 0 and 27 never loaded)
OUTD = 20           # output planes per core
HG, HPW = 32, 3     # h = hg*3 + hp
WPAD = 104          # 102 needed; padded to /4 so ACT memzero can bitcast u32
NPAIR = KK * KK     # 49 DoubleRow matmuls per output group
NOC = 5             # output groups of G=4 planes
G = 4
NCORES = 8
WS = 256.0          # fp8 weight scale; undone in the sigmoid
CHUNKS = [(1, 3), (4, 4), (8, 4), (12, 4), (16, 4), (20, 4), (24, 3)]
LOADD = 26          # planes 1..26 inclusive
NWARM = 96          # PE warm-up dummy matmuls before conv group 0

_prog_cache: dict = {}


def _build_banded(w: np.ndarray) -> np.ndarray:
    """w: [1,2,7,7,7] f32 -> lhsT [hin_pad 128, pair 49, c 2, hout 96] f32.

    out[o,h,w'] = sum_{c,kd,kw} lhsT[hq, kd*7+kw, c, h] * P[hq, c, o+1+kd, w'+kw]
    with P[h_in+3, stat, dp, w_in+3] = pooled stats (0=max, 1=sum) and
    lhsT[h+kh, kd*7+kw, c, h] = w[c,kd,kh,kw] * WS * (1/16 for c=1).
    """
    A = np.zeros((128, NPAIR, 2, 96), np.float32)
    hh = np.arange(96)
    for c in range(2):
        scale = WS if c == 0 else WS / C
        for kd in range(KK):
            for kw in range(KK):
                pair = kd * KK + kw
                for kh in range(KK):
                    A[hh + kh, pair, c, hh] = w[0, c, kd, kh, kw] * scale
    return A


def _build_program():
    import concourse.bass as bass  # noqa: F401
    import concourse.bacc as bacc
    import concourse.tile as tile
    from concourse import mybir
    from contextlib import ExitStack

    bf16 = mybir.dt.bfloat16
    fp8 = mybir.dt.float8e4

    nc = bacc.Bacc("TRN2")
    # xin/xout are host-pre-shuffled to the on-chip tile layout
    # (d, hg, t, c, hp, w) so loads and stores are fully contiguous
    # (~128 fat descriptors instead of 4096x576B -> ~10x cheaper
    # HWDGE issue and full line rate).
    xin = nc.dram_tensor(
        "xin", [LOADD, HG, 2, C, HPW, W], bf16, kind="ExternalInput"
    )
    wgt = nc.dram_tensor("wgt", [128, 2, NPAIR, 2, 96], fp8, kind="ExternalInput")
    xout = nc.dram_tensor(
        "xout", [OUTD, HG, 2, C, HPW, W], bf16, kind="ExternalOutput"
    )

    with tile.TileContext(nc) as tc, ExitStack() as ctx:
        singles = ctx.enter_context(tc.tile_pool(name="singles", bufs=1))
        lp = ctx.enter_context(tc.tile_pool(name="lp", bufs=5))
        trpool = ctx.enter_context(tc.tile_pool(name="tr", bufs=2))
        pspool = ctx.enter_context(tc.tile_pool(name="pstage", bufs=2))
        tpool = ctx.enter_context(tc.tile_pool(name="tmp", bufs=1))
        gpool = ctx.enter_context(tc.tile_pool(name="gate", bufs=3))
        gtpool = ctx.enter_context(tc.tile_pool(name="gateT", bufs=3))
        psum = ctx.enter_context(tc.tile_pool(name="psum", bufs=4, space="PSUM"))

        WG = singles.tile([128, 2, NPAIR, 2, 96], fp8)
        # Persistent pooled stats [hin_pad, tensor, stat(max,sum), dp, wp]
        P = singles.tile([128, 2, 2, DSLAB, WPAD], fp8)

        # ACT zeroes P (cheap, ACT is idle early; GpSimd memset would
        # grab the DVE-shared SBUF port). Weights load right behind on
        # the scalar ring; the sync ring drains input loads.
        nc.scalar.memzero(P[:].rearrange("p t s d w -> p t s (d w)"))
        nc.scalar.dma_start(out=WG[:], in_=wgt[:])

        ltiles: dict = {}

        def load_chunk(ic: int, split=False):
            p0, nd = CHUNKS[ic]
            np_ = nd * HG
            L = lp.tile([128, 2, C, HPW * W], bf16, tag="L")
            if split:  # two DMAs so tensor-f trees can start sooner
                for t in range(2):
                    src = xin[p0 - 1:p0 - 1 + nd, :, t].rearrange(
                        "d hg c hp w -> (d hg) c (hp w)"
                    )
                    nc.sync.dma_start(out=L[:np_, t], in_=src)
            else:
                src = xin[p0 - 1:p0 - 1 + nd].rearrange(
                    "d hg t c hp w -> (d hg) t c (hp w)"
                )
                nc.sync.dma_start(out=L[:np_], in_=src)
            ltiles[ic] = L

        def trees(ic: int):
            """Channel max+sum trees for both tensors on DVE, then stage
            the fp8 stats into P with one reshape DMA per plane."""
            p0, nd = CHUNKS[ic]
            np_ = nd * HG
            Lv = ltiles[ic][:np_]
            Tmax = trpool.tile([128, 2, 8, HPW * W], bf16, tag="Tmax")
            Tsum = trpool.tile([128, 2, 8, HPW * W], bf16, tag="Tsum")
            PS = pspool.tile([128, 2, 2, HPW, W], fp8, tag="PS")
            v = nc.vector
            v.tensor_max(Tmax[:np_], Lv[:, :, 0:8, :], Lv[:, :, 8:16, :])
            v.tensor_add(Tsum[:np_], Lv[:, :, 0:8, :], Lv[:, :, 8:16, :])
            v.tensor_max(Tmax[:np_, :, 0:4], Tmax[:np_, :, 0:4], Tmax[:np_, :, 4:8])
            v.tensor_add(Tsum[:np_, :, 0:4], Tsum[:np_, :, 0:4], Tsum[:np_, :, 4:8])
            v.tensor_max(Tmax[:np_, :, 0:2], Tmax[:np_, :, 0:2], Tmax[:np_, :, 2:4])
            v.tensor_add(Tsum[:np_, :, 0:2], Tsum[:np_, :, 0:2], Tsum[:np_, :, 2:4])
            PSm = PS[:np_, :, 0].rearrange("p t hp w -> p t (hp w)")
            PSs = PS[:np_, :, 1].rearrange("p t hp w -> p t (hp w)")
            v.tensor_max(PSm, Tmax[:np_, :, 0], Tmax[:np_, :, 1])
            v.tensor_add(PSs, Tsum[:np_, :, 0], Tsum[:np_, :, 1])
            return PS

        def trees_split(ic: int):
            """Like trees() but per-tensor ops, so tensor f's tree can
            start as soon as its half-load lands (head-latency cut)."""
            p0, nd = CHUNKS[ic]
            np_ = nd * HG
            Lv = ltiles[ic][:np_]
            Tmax = trpool.tile([128, 2, 8, HPW * W], bf16, tag="Tmax")
            Tsum = trpool.tile([128, 2, 8, HPW * W], bf16, tag="Tsum")
            PS = pspool.tile([128, 2, 2, HPW, W], fp8, tag="PS")
            v = nc.vector
            for t in range(2):
                Lt, Tm, Ts = Lv[:, t], Tmax[:np_, t], Tsum[:np_, t]
                v.tensor_max(Tm, Lt[:, 0:8, :], Lt[:, 8:16, :])
                v.tensor_add(Ts, Lt[:, 0:8, :], Lt[:, 8:16, :])
                v.tensor_max(Tm[:, 0:4], Tm[:, 0:4], Tm[:, 4:8])
                v.tensor_add(Ts[:, 0:4], Ts[:, 0:4], Ts[:, 4:8])
                v.tensor_max(Tm[:, 0:2], Tm[:, 0:2], Tm[:, 2:4])
                v.tensor_add(Ts[:, 0:2], Ts[:, 0:2], Ts[:, 2:4])
                PSm = PS[:np_, t, 0].rearrange("p hp w -> p (hp w)")
                PSs = PS[:np_, t, 1].rearrange("p hp w -> p (hp w)")
                v.tensor_max(PSm, Tm[:, 0], Tm[:, 1])
                v.tensor_add(PSs, Ts[:, 0], Ts[:, 1])
            return PS

        def pstage(ic: int, PS, engines=(nc.sync, nc.scalar)):
            # stage into P: per (plane, hp): src partitions d*32+hg, free
            # (t,s,w); dst partitions 3+hp+3*hg (step 3). Both sides merge
            # (t,s) so the DMA balancer sees 3 dims. Early chunks split
            # across both HWDGE queues (issue ~0.7us each); chunks 4-6 go
            # sync-only so gate reshapes never queue behind P batches on
            # the scalar ring (that head-of-line starved DVE ~18us/group).
            p0, nd = CHUNKS[ic]
            for d in range(nd):
                for hp in range(HPW):
                    eng = engines[(d * HPW + hp) % len(engines)]
                    eng.dma_start(
                        out=P[3 + hp:3 + hp + 94:3, :, :, p0 + d, 3:3 + W],
                        in_=PS[d * HG:(d + 1) * HG, :, :, hp, :],
                    )

        def conv_group(oc: int):
            o0 = G * oc
            gate = gpool.tile([96, 2, G, 96], bf16, tag="gate")
            gateT = gtpool.tile([128, 2, HPW, W], bf16, tag="gT")
            for t in range(2):
                acc = psum.tile([96, G, 96], mybir.dt.float32, tag="acc")
                for kd in range(KK):
                    dsl = slice(o0 + 1 + kd, o0 + 1 + kd + G)
                    for kw in range(KK):
                        nc.tensor.matmul(
                            acc[:],
                            WG[:, t, kd * KK + kw],
                            P[:, t, 0:2, dsl, kw:kw + 96],
                            start=(kd == 0 and kw == 0),
                            stop=(kd == KK - 1 and kw == KK - 1),
                            perf_mode=mybir.MatmulPerfMode.DoubleRow,
                        )
                nc.scalar.activation(
                    out=gate[:, t], in_=acc[:],
                    func=mybir.ActivationFunctionType.Sigmoid,
                    scale=1.0 / WS,
                )
                # reshape this tensor's gate immediately: gating's first
                # ops need only the f half, so they overlap the m conv
                for d in range(G):
                    nc.scalar.dma_start(
                        out=gateT[d * HG:(d + 1) * HG, t],
                        in_=gate[:, t, d, :],
                    )
            return gateT

        def gating(oc: int, gateT):
            L = ltiles[oc + 1]
            gf = (
                gateT[:, 0].rearrange("p hp w -> p (hp w)").unsqueeze(1)
                .broadcast_to((128, C, HPW * W))
            )
            gm = (
                gateT[:, 1].rearrange("p hp w -> p (hp w)").unsqueeze(1)
                .broadcast_to((128, C, HPW * W))
            )
            # products into a temp, then add in place into L: fo lands in
            # the fix slot, mo in the move slot; the store reads L itself.
            # Per-tensor stores so the fo store overlaps the mo add.
            T = tpool.tile([128, 2, C, HPW * W], bf16, tag="T")
            v = nc.vector
            # broadcast operand in slot 0 (slot-1 broadcast is ~2x slower)
            v.tensor_mul(T[:, 0], gf, L[:, 1])
            v.tensor_mul(T[:, 1], gm, L[:, 0])
            dst = xout[G * oc:G * oc + G]
            v.tensor_add(L[:, 0], T[:, 0], L[:, 0])
            nc.sync.dma_start(
                out=dst[:, :, 0].rearrange("d hg c hp w -> (d hg) c (hp w)"),
                in_=L[:, 0],
            )
            v.tensor_add(L[:, 1], T[:, 1], L[:, 1])
            nc.sync.dma_start(
                out=dst[:, :, 1].rearrange("d hg c hp w -> (d hg) c (hp w)"),
                in_=L[:, 1],
            )

        # Software pipeline. DVE queue order: t0..t5, g0, t6, g1..g4
        # (all trees lead their gating consumers so P staging never
        # trails the PE and the cross-engine gate waits never
        # head-of-line block tree work). Sync queue issue order is
        # monotone in semaphore-resolution time:
        #   L0-3, P0[t0], P1[t1], L4, P2[t2], L5[t0/buf], P3[t3],
        #   P4[t4], P5[t5], S0[g0], L6[g0/buf], P6[t6], S1[g1], S2-4.
        load_chunk(0, split=True)
        load_chunk(1)
        load_chunk(2)
        load_chunk(3)
        ps0 = trees_split(0)
        ps1 = trees(1)
        pstage(0, ps0)
        pstage(1, ps1)
        load_chunk(4)
        ps2 = trees(2)
        pstage(2, ps2)
        load_chunk(5)  # reuses chunk 0's buffer (freed after trees 0)
        ps3 = trees(3)
        pstage(3, ps3)
        gts = {0: conv_group(0)}
        ps4 = trees(4)
        pstage(4, ps4, engines=(nc.sync,))
        ps5 = trees(5)
        pstage(5, ps5, engines=(nc.sync,))
        gating(0, gts[0])
        load_chunk(6)  # reuses chunk 1's buffer; gating(0) read it above
        gts[1] = conv_group(1)
        ps6 = trees(6)
        pstage(6, ps6, engines=(nc.sync,))
        gating(1, gts[1])
        gts[2] = conv_group(2)
        gating(2, gts[2])
        gts[3] = conv_group(3)
        gating(3, gts[3])
        gts[4] = conv_group(4)
        gating(4, gts[4])

    nc.compile()
    return nc


def _get_program():
    if "nc" not in _prog_cache:
        _prog_cache["nc"] = _build_program()
    return _prog_cache["nc"]


def _shard(fix, move, Wm):
    in_maps = []
    for core in range(NCORES):
        b, dq = core // 4, core % 4
        lo = 20 * dq - 3  # global index of slab plane 1
        s0, s1 = max(lo, 0), min(lo + LOADD, D)
        slab = np.zeros((2, C, LOADD, H, W), ml_dtypes.bfloat16)
        slab[0, :, s0 - lo:s1 - lo] = fix[b, :, s0:s1]
        slab[1, :, s0 - lo:s1 - lo] = move[b, :, s0:s1]
        # host-side shuffle to the on-chip layout (d, hg, t, c, hp, w)
        shuf = np.ascontiguousarray(
            slab.reshape(2, C, LOADD, HG, HPW, W).transpose(2, 3, 0, 1, 4, 5)
        )
        in_maps.append({"xin": shuf, "wgt": Wm})
    return in_maps


def kernel(fix, move, w_f2m, w_m2f, __trace=False):
    fix = np.asarray(fix, dtype=np.float32).astype(ml_dtypes.bfloat16)
    move = np.asarray(move, dtype=np.float32).astype(ml_dtypes.bfloat16)
    Af = _build_banded(np.asarray(w_f2m, dtype=np.float32))
    Am = _build_banded(np.asarray(w_m2f, dtype=np.float32))
    Wm = np.ascontiguousarray(
        np.stack([Af, Am]).transpose(1, 0, 2, 3, 4)
    ).astype(ml_dtypes.float8_e4m3fn)

    nc = _get_program()
    in_maps = _shard(fix, move, Wm)

    from concourse.bass_utils import run_bass_kernel_spmd

    res = run_bass_kernel_spmd(
        nc, in_maps, core_ids=list(range(NCORES)), trace=__trace
    )
    _prog_cache["last_results"] = res

    fix_out = np.empty((B, C, D, H, W), np.float32)
    move_out = np.empty((B, C, D, H, W), np.float32)
    for core in range(NCORES):
        b, dq = core // 4, core % 4
        out = res.results[core]["xout"]  # [d, hg, t, c, hp, w]
        out = out.transpose(2, 3, 0, 1, 4, 5).reshape(
            2, C, OUTD, H, W
        ).astype(np.float32)
        fix_out[b, :, 20 * dq:20 * dq + 20] = out[0]
        move_out[b, :, 20 * dq:20 * dq + 20] = out[1]
    return fix_out, move_out


# revision 33
# speedup vs baseline: 1.1130x; 1.0461x over previous
"""Trainium2 Bass kernel for nn_CSABlock (dual spatial-attention gating).

Reference computation:
    sa_x  = sigmoid(conv3d(concat[max_c(x), mean_c(x)], w, k=7, pad=3))
    fix_out  = move * sa_fix + fix
    move_out = fix * sa_move + move

Sharding: 8 cores = (batch 2) x (D quarters of 20 planes). Each core gets a
26-plane input slab (3-voxel halo each side) per tensor in bf16 and produces
20 output planes in bf16; the host casts f32<->bf16 and shards/gathers.

v6 design notes (evidence from the v5 trace):
  - v5 ran DVE and GpSimd tensor_tensor streams concurrently. They arbitrate
    an exclusive SBUF shared-port lock, so they serialize: DVE ops overlapped
    by GpSimd measured 3.0x their cost-model time vs 1.07x when alone.
    v6 therefore runs ALL elementwise work on DVE (2 elem/cyc bf16) and
    leaves GpSimd idle; ACT only does memzero + sigmoid.
  - fix/move ride one extra tensor dim everywhere (loads, trees, stats,
    gates, gating, stores), halving instruction and DMA counts.
  - All DMA is HWDGE. sync queue: loads, P-stage, stores (issue order
    matches semaphore resolution order, so no head-of-line stalls).
    scalar queue: weights + gate reshapes (right after their sigmoids).
  - Dummy matmuls on the weight tile bridge the PE-idle gap before the
    first conv group so HAM doesn't drop the PE clock to 1.2 GHz.

Per-core pipeline:
  - Stream D in chunks (3,4,4,4,4,4,3 planes). Load tile layout:
    partition=(d,hg32), free=(t2, c16, hp3*w96) bf16 -> 576B lines.
  - Channel max/sum trees on DVE (mean's 1/16 folded into conv weights);
    final tree level writes fp8 stats.
  - Stats staged into persistent P [hin_pad128, t2, stat2, dp28, wp102]
    fp8 via per-d SBUF->SBUF reshape DMAs.
  - Conv: 49 fp8 DoubleRow matmuls per 4-plane output group per tensor;
    stats ride the k-tile dim; contraction over padded H with banded
    weights lhsT[hin, t, (kd,kw), c, hout] scaled by WS=256.
  - Sigmoid on ACT with scale=1/WS (PSUM -> SBUF bf16), reshaped to the
    data layout via 4 small DMAs per group.
  - Gating all-bf16 on DVE, one 16-channel op per (tensor, mul/add).
"""

import sys

import numpy as np

for _p in ("/opt/trn_rl_repo",):
    if _p not in sys.path:
        sys.path.insert(0, _p)

import ml_dtypes  # noqa: E402

B, C, D, H, W = 2, 16, 80, 96, 96
KK = 7
DSLAB = 28          # slab plane indexing (plane 0 and 27 never loaded)
OUTD = 20           # output planes per core
HG, HPW = 32, 3     # h = hg*3 + hp
WPAD = 104          # 102 needed; padded to /4 so ACT memzero can bitcast u32
NPAIR = KK * KK     # 49 DoubleRow matmuls per output group
NOC = 5             # output groups of G=4 planes
G = 4
NCORES = 8
WS = 256.0          # fp8 weight scale; undone in the sigmoid
CHUNKS = [(1, 3), (4, 4), (8, 4), (12, 4), (16, 4), (20, 4), (24, 3)]
LOADD = 26          # planes 1..26 inclusive
NWARM = 96          # PE warm-up dummy matmuls before conv group 0

_prog_cache: dict = {}


def _build_banded(w: np.ndarray) -> np.ndarray:
    """w: [1,2,7,7,7] f32 -> lhsT [hin_pad 128, pair 49, c 2, hout 96] f32.

    out[o,h,w'] = sum_{c,kd,kw} lhsT[hq, kd*7+kw, c, h] * P[hq, c, o+1+kd, w'+kw]
    with P[h_in+3, stat, dp, w_in+3] = pooled stats (0=max, 1=sum) and
    lhsT[h+kh, kd*7+kw, c, h] = w[c,kd,kh,kw] * WS * (1/16 for c=1).
    """
    A = np.zeros((128, NPAIR, 2, 96), np.float32)
    hh = np.arange(96)
    for c in range(2):
        scale = WS if c == 0 else WS / C
        for kd in range(KK):
            for kw in range(KK):
                pair = kd * KK + kw
                for kh in range(KK):
                    A[hh + kh, pair, c, hh] = w[0, c, kd, kh, kw] * scale
    return A


def _build_program():
    import concourse.bass as bass  # noqa: F401
    import concourse.bacc as bacc
    import concourse.tile as tile
    from concourse import mybir
    from contextlib import ExitStack

    bf16 = mybir.dt.bfloat16
    fp8 = mybir.dt.float8e4

    nc = bacc.Bacc("TRN2")
    # xin/xout are host-pre-shuffled to the on-chip tile layout
    # (d, hg, t, c, hp, w) so loads and stores are fully contiguous
    # (~128 fat descriptors instead of 4096x576B -> ~10x cheaper
    # HWDGE issue and full line rate).
    xin = nc.dram_tensor(
        "xin", [LOADD, HG, 2, C, HPW, W], bf16, kind="ExternalInput"
    )
    wgt = nc.dram_tensor("wgt", [128, 2, NPAIR, 2, 96], fp8, kind="ExternalInput")
    xout = nc.dram_tensor(
        "xout", [OUTD, HG, 2, C, HPW, W], bf16, kind="ExternalOutput"
    )

    with tile.TileContext(nc) as tc, ExitStack() as ctx:
        singles = ctx.enter_context(tc.tile_pool(name="singles", bufs=1))
        lp = ctx.enter_context(tc.tile_pool(name="lp", bufs=5))
        trpool = ctx.enter_context(tc.tile_pool(name="tr", bufs=2))
        pspool = ctx.enter_context(tc.tile_pool(name="pstage", bufs=2))
        tpool = ctx.enter_context(tc.tile_pool(name="tmp", bufs=1))
        x6pool = ctx.enter_context(tc.tile_pool(name="x6", bufs=1))
        gpool = ctx.enter_context(tc.tile_pool(name="gate", bufs=3))
        gtpool = ctx.enter_context(tc.tile_pool(name="gateT", bufs=3))
        psum = ctx.enter_context(tc.tile_pool(name="psum", bufs=4, space="PSUM"))

        WG = singles.tile([128, 2, NPAIR, 2, 96], fp8)
        # Persistent pooled stats [hin_pad, tensor, stat(max,sum), dp, wp]
        P = singles.tile([128, 2, 2, DSLAB, WPAD], fp8)

        # ACT zeroes P (cheap, ACT is idle early; GpSimd memset would
        # grab the DVE-shared SBUF port). Weights load right behind on
        # the scalar ring; the sync ring drains input loads.
        nc.scalar.memzero(P[:].rearrange("p t s d w -> p t s (d w)"))
        nc.scalar.dma_start(out=WG[:], in_=wgt[:])

        ltiles: dict = {}

        def load_chunk(ic: int, split=False):
            p0, nd = CHUNKS[ic]
            np_ = nd * HG
            L = lp.tile([128, 2, C, HPW * W], bf16, tag="L")
            if split:  # two DMAs so tensor-f trees can start sooner
                for t in range(2):
                    src = xin[p0 - 1:p0 - 1 + nd, :, t].rearrange(
                        "d hg c hp w -> (d hg) c (hp w)"
                    )
                    nc.sync.dma_start(out=L[:np_, t], in_=src)
            else:
                src = xin[p0 - 1:p0 - 1 + nd].rearrange(
                    "d hg t c hp w -> (d hg) t c (hp w)"
                )
                nc.sync.dma_start(out=L[:np_], in_=src)
            ltiles[ic] = L

        def trees(ic: int):
            """Channel max+sum trees for both tensors on DVE, then stage
            the fp8 stats into P with one reshape DMA per plane."""
            p0, nd = CHUNKS[ic]
            np_ = nd * HG
            Lv = ltiles[ic][:np_]
            Tmax = trpool.tile([128, 2, 8, HPW * W], bf16, tag="Tmax")
            Tsum = trpool.tile([128, 2, 8, HPW * W], bf16, tag="Tsum")
            PS = pspool.tile([128, 2, 2, HPW, W], fp8, tag="PS")
            v = nc.vector
            v.tensor_max(Tmax[:np_], Lv[:, :, 0:8, :], Lv[:, :, 8:16, :])
            v.tensor_add(Tsum[:np_], Lv[:, :, 0:8, :], Lv[:, :, 8:16, :])
            v.tensor_max(Tmax[:np_, :, 0:4], Tmax[:np_, :, 0:4], Tmax[:np_, :, 4:8])
            v.tensor_add(Tsum[:np_, :, 0:4], Tsum[:np_, :, 0:4], Tsum[:np_, :, 4:8])
            v.tensor_max(Tmax[:np_, :, 0:2], Tmax[:np_, :, 0:2], Tmax[:np_, :, 2:4])
            v.tensor_add(Tsum[:np_, :, 0:2], Tsum[:np_, :, 0:2], Tsum[:np_, :, 2:4])
            PSm = PS[:np_, :, 0].rearrange("p t hp w -> p t (hp w)")
            PSs = PS[:np_, :, 1].rearrange("p t hp w -> p t (hp w)")
            v.tensor_max(PSm, Tmax[:np_, :, 0], Tmax[:np_, :, 1])
            v.tensor_add(PSs, Tsum[:np_, :, 0], Tsum[:np_, :, 1])
            return PS

        def trees_split(ic: int):
            """Like trees() but per-tensor ops, so tensor f's tree can
            start as soon as its half-load lands (head-latency cut)."""
            p0, nd = CHUNKS[ic]
            np_ = nd * HG
            Lv = ltiles[ic][:np_]
            Tmax = trpool.tile([128, 2, 8, HPW * W], bf16, tag="Tmax")
            Tsum = trpool.tile([128, 2, 8, HPW * W], bf16, tag="Tsum")
            PS = pspool.tile([128, 2, 2, HPW, W], fp8, tag="PS")
            v = nc.vector
            for t in range(2):
                Lt, Tm, Ts = Lv[:, t], Tmax[:np_, t], Tsum[:np_, t]
                v.tensor_max(Tm, Lt[:, 0:8, :], Lt[:, 8:16, :])
                v.tensor_add(Ts, Lt[:, 0:8, :], Lt[:, 8:16, :])
                v.tensor_max(Tm[:, 0:4], Tm[:, 0:4], Tm[:, 4:8])
                v.tensor_add(Ts[:, 0:4], Ts[:, 0:4], Ts[:, 4:8])
                v.tensor_max(Tm[:, 0:2], Tm[:, 0:2], Tm[:, 2:4])
                v.tensor_add(Ts[:, 0:2], Ts[:, 0:2], Ts[:, 2:4])
                PSm = PS[:np_, t, 0].rearrange("p hp w -> p (hp w)")
                PSs = PS[:np_, t, 1].rearrange("p hp w -> p (hp w)")
                v.tensor_max(PSm, Tm[:, 0], Tm[:, 1])
                v.tensor_add(PSs, Ts[:, 0], Ts[:, 1])
            return PS

        def chunk6_half(t: int, X6, PS):
            p0, nd = CHUNKS[6]
            np_ = nd * HG
            nc.sync.dma_start(
                out=X6[:np_],
                in_=xin[p0 - 1:p0 - 1 + nd, :, t].rearrange(
                    "d hg c hp w -> (d hg) c (hp w)"
                ),
            )
            Xv = X6[:np_]
            Tmax = trpool.tile([128, 2, 8, HPW * W], bf16, tag="Tmax")
            Tsum = trpool.tile([128, 2, 8, HPW * W], bf16, tag="Tsum")
            v = nc.vector
            Tm, Ts = Tmax[:np_, 0], Tsum[:np_, 0]
            v.tensor_max(Tm, Xv[:, 0:8, :], Xv[:, 8:16, :])
            v.tensor_add(Ts, Xv[:, 0:8, :], Xv[:, 8:16, :])
            v.tensor_max(Tm[:, 0:4], Tm[:, 0:4], Tm[:, 4:8])
            v.tensor_add(Ts[:, 0:4], Ts[:, 0:4], Ts[:, 4:8])
            v.tensor_max(Tm[:, 0:2], Tm[:, 0:2], Tm[:, 2:4])
            v.tensor_add(Ts[:, 0:2], Ts[:, 0:2], Ts[:, 2:4])
            PSm = PS[:np_, t, 0].rearrange("p hp w -> p (hp w)")
            PSs = PS[:np_, t, 1].rearrange("p hp w -> p (hp w)")
            v.tensor_max(PSm, Tm[:, 0], Tm[:, 1])
            v.tensor_add(PSs, Ts[:, 0], Ts[:, 1])
            for d in range(nd):
                for hp in range(HPW):
                    eng = (nc.sync, nc.scalar)[(d * HPW + hp) % 2]
                    eng.dma_start(
                        out=P[3 + hp:3 + hp + 94:3, t, :, p0 + d, 3:3 + W],
                        in_=PS[d * HG:(d + 1) * HG, t, :, hp, :],
                    )

        def pstage(ic: int, PS, engines=(nc.sync, nc.scalar)):
            # stage into P: per (plane, hp): src partitions d*32+hg, free
            # (t,s,w); dst partitions 3+hp+3*hg (step 3). Both sides merge
            # (t,s) so the DMA balancer sees 3 dims. Early chunks split
            # across both HWDGE queues (issue ~0.7us each); chunks 4-6 go
            # sync-only so gate reshapes never queue behind P batches on
            # the scalar ring (that head-of-line starved DVE ~18us/group).
            p0, nd = CHUNKS[ic]
            for d in range(nd):
                for hp in range(HPW):
                    eng = engines[(d * HPW + hp) % len(engines)]
                    eng.dma_start(
                        out=P[3 + hp:3 + hp + 94:3, :, :, p0 + d, 3:3 + W],
                        in_=PS[d * HG:(d + 1) * HG, :, :, hp, :],
                    )

        def conv_group(oc: int):
            o0 = G * oc
            gate = gpool.tile([96, 2, G, 96], bf16, tag="gate")
            gateT = gtpool.tile([128, 2, HPW, W], bf16, tag="gT")
            for t in range(2):
                acc = psum.tile([96, G, 96], mybir.dt.float32, tag="acc")
                for kd in range(KK):
                    dsl = slice(o0 + 1 + kd, o0 + 1 + kd + G)
                    for kw in range(KK):
                        nc.tensor.matmul(
                            acc[:],
                            WG[:, t, kd * KK + kw],
                            P[:, t, 0:2, dsl, kw:kw + 96],
                            start=(kd == 0 and kw == 0),
                            stop=(kd == KK - 1 and kw == KK - 1),
                            perf_mode=mybir.MatmulPerfMode.DoubleRow,
                        )
                nc.scalar.activation(
                    out=gate[:, t], in_=acc[:],
                    func=mybir.ActivationFunctionType.Sigmoid,
                    scale=1.0 / WS,
                )
                # reshape this tensor's gate immediately: gating's first
                # ops need only the f half, so they overlap the m conv
                for d in range(G):
                    nc.scalar.dma_start(
                        out=gateT[d * HG:(d + 1) * HG, t],
                        in_=gate[:, t, d, :],
                    )
            return gateT

        def gating(oc: int, gateT):
            L = ltiles[oc + 1]
            gf = (
                gateT[:, 0].rearrange("p hp w -> p (hp w)").unsqueeze(1)
                .broadcast_to((128, C, HPW * W))
            )
            gm = (
                gateT[:, 1].rearrange("p hp w -> p (hp w)").unsqueeze(1)
                .broadcast_to((128, C, HPW * W))
            )
            # products into a temp, then add in place into L: fo lands in
            # the fix slot, mo in the move slot; the store reads L itself.
            # Per-tensor stores so the fo store overlaps the mo add.
            T = tpool.tile([128, 2, C, HPW * W], bf16, tag="T")
            v = nc.vector
            # broadcast operand in slot 0 (slot-1 broadcast is ~2x slower)
            v.tensor_mul(T[:, 0], gf, L[:, 1])
            v.tensor_mul(T[:, 1], gm, L[:, 0])
            dst = xout[G * oc:G * oc + G]
            v.tensor_add(L[:, 0], T[:, 0], L[:, 0])
            nc.sync.dma_start(
                out=dst[:, :, 0].rearrange("d hg c hp w -> (d hg) c (hp w)"),
                in_=L[:, 0],
            )
            v.tensor_add(L[:, 1], T[:, 1], L[:, 1])
            nc.sync.dma_start(
                out=dst[:, :, 1].rearrange("d hg c hp w -> (d hg) c (hp w)"),
                in_=L[:, 1],
            )

        # Software pipeline. DVE queue order: t0..t5, g0, t6, g1..g4
        # (all trees lead their gating consumers so P staging never
        # trails the PE and the cross-engine gate waits never
        # head-of-line block tree work). Sync queue issue order is
        # monotone in semaphore-resolution time:
        #   L0-3, P0[t0], P1[t1], L4, P2[t2], L5[t0/buf], P3[t3],
        #   P4[t4], P5[t5], S0[g0], L6[g0/buf], P6[t6], S1[g1], S2-4.
        load_chunk(0, split=True)
        load_chunk(1)
        load_chunk(2)
        load_chunk(3)
        ps0 = trees_split(0)
        ps1 = trees(1)
        pstage(0, ps0)
        pstage(1, ps1)
        load_chunk(4)
        ps2 = trees(2)
        pstage(2, ps2)
        load_chunk(5)  # reuses chunk 0's buffer (freed after trees 0)
        ps3 = trees(3)
        pstage(3, ps3)
        gts = {0: conv_group(0)}
        # chunk 6 is tree-only: stream it through a dedicated half-size
        # buffer per tensor so its stats stage ~50us earlier than waiting
        # for an L buffer (which only frees after store 0). This removes
        # a 20us DVE stall and the conv-4 PE starvation.
        X6 = x6pool.tile([128, C, HPW * W], bf16, tag="X6")
        ps6 = pspool.tile([128, 2, 2, HPW, W], fp8, tag="PS")
        chunk6_half(0, X6, ps6)
        ps4 = trees(4)
        pstage(4, ps4, engines=(nc.sync,))
        chunk6_half(1, X6, ps6)
        ps5 = trees(5)
        pstage(5, ps5, engines=(nc.sync,))
        gating(0, gts[0])
        gts[1] = conv_group(1)
        gating(1, gts[1])
        gts[2] = conv_group(2)
        gating(2, gts[2])
        gts[3] = conv_group(3)
        gating(3, gts[3])
        gts[4] = conv_group(4)
        gating(4, gts[4])

    nc.compile()
    return nc


def _get_program():
    if "nc" not in _prog_cache:
        _prog_cache["nc"] = _build_program()
    return _prog_cache["nc"]


def _shard(fix, move, Wm):
    in_maps = []
    for core in range(NCORES):
        b, dq = core // 4, core % 4
        lo = 20 * dq - 3  # global index of slab plane 1
        s0, s1 = max(lo, 0), min(lo + LOADD, D)
        slab = np.zeros((2, C, LOADD, H, W), ml_dtypes.bfloat16)
        slab[0, :, s0 - lo:s1 - lo] = fix[b, :, s0:s1]
        slab[1, :, s0 - lo:s1 - lo] = move[b, :, s0:s1]
        # host-side shuffle to the on-chip layout (d, hg, t, c, hp, w)
        shuf = np.ascontiguousarray(
            slab.reshape(2, C, LOADD, HG, HPW, W).transpose(2, 3, 0, 1, 4, 5)
        )
        in_maps.append({"xin": shuf, "wgt": Wm})
    return in_maps


def kernel(fix, move, w_f2m, w_m2f, __trace=False):
    fix = np.asarray(fix, dtype=np.float32).astype(ml_dtypes.bfloat16)
    move = np.asarray(move, dtype=np.float32).astype(ml_dtypes.bfloat16)
    Af = _build_banded(np.asarray(w_f2m, dtype=np.float32))
    Am = _build_banded(np.asarray(w_m2f, dtype=np.float32))
    Wm = np.ascontiguousarray(
        np.stack([Af, Am]).transpose(1, 0, 2, 3, 4)
    ).astype(ml_dtypes.float8_e4m3fn)

    nc = _get_program()
    in_maps = _shard(fix, move, Wm)

    from concourse.bass_utils import run_bass_kernel_spmd

    res = run_bass_kernel_spmd(
        nc, in_maps, core_ids=list(range(NCORES)), trace=__trace
    )
    _prog_cache["last_results"] = res

    fix_out = np.empty((B, C, D, H, W), np.float32)
    move_out = np.empty((B, C, D, H, W), np.float32)
    for core in range(NCORES):
        b, dq = core // 4, core % 4
        out = res.results[core]["xout"]  # [d, hg, t, c, hp, w]
        out = out.transpose(2, 3, 0, 1, 4, 5).reshape(
            2, C, OUTD, H, W
        ).astype(np.float32)
        fix_out[b, :, 20 * dq:20 * dq + 20] = out[0]
        move_out[b, :, 20 * dq:20 * dq + 20] = out[1]
    return fix_out, move_out
